# revision 1
# baseline (speedup 1.0000x reference)
"""L2 self-attention (q==k) Bass/Tile kernel for 8 TRN2 NeuronCores.

Sharding: core c = 2*b + g handles batch b and head-group g (8 of 16 heads).
Each core computes the partial output  attn_out_bg @ Wo[g*512:(g+1)*512, :].
Host sums the two partials per batch and adds bo.

Math per head (s = DIM_HEAD**-0.5):
  sim_ij = -s*||q_i - q_j||^2 = 2s*AB_ij - s*AA_i - s*AA_j
  softmax rows are invariant to the per-row constant  -s*AA_i, so
  P_ij = exp(2s*AB_ij - s*AA_j) / sum_j exp(2s*AB_ij - s*AA_j)
  Exponent is <= 0 + bounded (2ab <= a^2+b^2), no max-subtraction needed.
  out = P @ v ; den folded in as a 65th (ones) column of v.
"""

import numpy as np

B, N, D = 4, 2048, 1024
HEADS, DIM_HEAD = 16, 64
INNER = HEADS * DIM_HEAD
SCALE = DIM_HEAD ** -0.5

NCORES = 8
NH = 8            # heads per core
DL = NH * DIM_HEAD  # 512 local inner dims
KT = D // 128     # 8 full contraction tiles for projections
JT = N // 128     # 16 key tiles
IC = 2            # i-chunks of 1024 columns
ICW = N // IC     # 1024

_CACHE = {}


def _build_nc():
    import concourse.bacc as bacc
    import concourse.bass as bass
    import concourse.mybir as mybir
    import concourse.tile as tile
    from concourse.masks import make_identity

    f32 = mybir.dt.float32
    f32r = mybir.dt.float32r
    X = mybir.AxisListType.X
    EXP = mybir.ActivationFunctionType.Exp

    nc = bacc.Bacc("TRN2", target_bir_lowering=False, debug=False,
                   num_devices=NCORES)

    xTa_d = nc.dram_tensor("xTa", [D + 1, N], f32r, kind="ExternalInput")
    wqa_d = nc.dram_tensor("wqa", [D + 1, DL], f32r, kind="ExternalInput")
    wva_d = nc.dram_tensor("wva", [D + 1, DL], f32r, kind="ExternalInput")
    wo_d = nc.dram_tensor("wo", [DL, D], f32r, kind="ExternalInput")
    part_d = nc.dram_tensor("part", [N, D], f32, kind="ExternalOutput")
    xTa = xTa_d.ap()
    wqa = wqa_d.ap()
    wva = wva_d.ap()
    wo_ap = wo_d.ap()
    part = part_d.ap()

    with tile.TileContext(nc) as tc, \
         tc.tile_pool(name="persist", bufs=1) as persist:
        # ---- persistent tensors (whole-kernel lifetime) ----
        qT = [persist.tile([128, N], f32r, tag=f"qT{t}", name=f"qT{t}")
              for t in range(4)]
        v_aug = persist.tile([128, JT, NH * 65], f32r, tag="v_aug", name="v_aug")
        aa = persist.tile([128, NH * JT], f32, tag="aa", name="aa")
        ident = persist.tile([128, 128], f32, tag="ident", name="ident")

        ones1 = persist.tile([128, JT, 1], f32, tag="ones1", name="ones1")
        make_identity(nc, ident)
        nc.vector.tensor_scalar_mul(ident, ident, -SCALE)
        nc.vector.memset(ones1, 1.0)
        for h in range(NH):
            nc.vector.tensor_copy(v_aug[:, :, h * 65 + 64 : h * 65 + 65], ones1)

        # ---- phase 1: projections ----
        with tc.tile_pool(name="pin", bufs=1) as pin:
            xt = [pin.tile([128, N], f32r, tag=f"xt{k}", name=f"xt{k}") for k in range(KT)]
            xt.append(pin.tile([1, N], f32r, tag="xt_ones", name="xt_ones"))
            wq = [pin.tile([128, DL], f32r, tag=f"wq{k}", name=f"wq{k}") for k in range(KT)]
            wq.append(pin.tile([1, DL], f32r, tag="wq_b", name="wq_b"))
            wv = [pin.tile([128, DL], f32r, tag=f"wv{k}", name=f"wv{k}") for k in range(KT)]
            wv.append(pin.tile([1, DL], f32r, tag="wv_b", name="wv_b"))
            for k in range(KT):
                sl = slice(k * 128, (k + 1) * 128)
                nc.sync.dma_start(out=xt[k], in_=xTa[sl, :])
                nc.sync.dma_start(out=wq[k], in_=wqa[sl, :])
                nc.sync.dma_start(out=wv[k], in_=wva[sl, :])
            nc.sync.dma_start(out=xt[KT], in_=xTa[D : D + 1, :])
            nc.sync.dma_start(out=wq[KT], in_=wqa[D : D + 1, :])
            nc.sync.dma_start(out=wv[KT], in_=wva[D : D + 1, :])

            # qT[d, i] : lhsT = wqa[:, d-tile], rhs = xTa[:, i-chunk]
            with tc.tile_pool(name="qps", bufs=2, space="PSUM") as qps:
                for dt in range(4):
                    ps = qps.tile([128, N], f32, tag="qproj")
                    dsl = slice(dt * 128, (dt + 1) * 128)
                    for k in range(KT + 1):
                        for nck in range(4):
                            nsl = slice(nck * 512, (nck + 1) * 512)
                            nc.tensor.matmul(ps[:, nsl], lhsT=wq[k][:, dsl],
                                             rhs=xt[k][:, nsl],
                                             start=(k == 0), stop=(k == KT))
                    nc.vector.tensor_copy(qT[dt], ps)

            # v[i, d] : lhsT = xTa[:, i-tile], rhs = wva ; scatter into v_aug
            with tc.tile_pool(name="vps", bufs=4, space="PSUM") as vps:
                for it in range(JT):
                    ps = vps.tile([128, DL], f32, tag="vproj")
                    isl = slice(it * 128, (it + 1) * 128)
                    for k in range(KT + 1):
                        nc.tensor.matmul(ps, lhsT=xt[k][:, isl], rhs=wv[k],
                                         start=(k == 0), stop=(k == KT))
                    src = ps.rearrange("p (h w) -> p h w", w=64)
                    dst = v_aug[:, it, :].rearrange("p (h w) -> p h w", w=65)
                    nc.vector.tensor_copy(dst[:, :, 0:64], src)

        # allocated after the projection pool closes so phase-1 SBUF peak
        # (xt/wq/wv tiles) and these never coexist in the address map
        p2 = tc.alloc_tile_pool(name="persist2", bufs=1)
        ot = [p2.tile([128, N], f32r, tag=f"ot{t}", name=f"ot{t}")
              for t in range(4)]
        wo_sb = [p2.tile([128, D], f32r, tag=f"wo{t}", name=f"wo{t}")
                 for t in range(4)]
        for t in range(4):
            nc.sync.dma_start(out=wo_sb[t], in_=wo_ap[t * 128 : (t + 1) * 128, :])

        # ---- phase 2a: AA diag pass:  aa[:, h*JT+jt] = -s * ||q_j||^2 ----
        with tc.tile_pool(name="dps", bufs=4, space="PSUM") as dps, \
             tc.tile_pool(name="dsb", bufs=4) as dsb:
            for h in range(NH):
                dt, half = divmod(h, 2)
                rows = slice(half * 64, half * 64 + 64)
                for jt in range(JT):
                    jsl = slice(jt * 128, (jt + 1) * 128)
                    ps = dps.tile([128, 128], f32, tag="diag")
                    nc.tensor.matmul(ps, lhsT=qT[dt][rows, jsl],
                                     rhs=qT[dt][rows, jsl],
                                     start=True, stop=True)
                    sc = dsb.tile([128, 128], f32, tag="dsc")
                    nc.vector.tensor_mul(sc, ps, ident)
                    col = h * JT + jt
                    nc.vector.reduce_sum(out=aa[:, col : col + 1], in_=sc,
                                         axis=X)

        # ---- phase 2b: attention per head ----
        with tc.tile_pool(name="sps", bufs=2, space="PSUM") as sps, \
             tc.tile_pool(name="nps", bufs=2, space="PSUM") as nps, \
             tc.tile_pool(name="gp", bufs=3) as gp, \
             tc.tile_pool(name="nrm", bufs=2) as nrm:
            for h in range(NH):
                dt, half = divmod(h, 2)
                rows = slice(half * 64, half * 64 + 64)
                vsl = slice(h * 65, (h + 1) * 65)
                for ic in range(IC):
                    i0 = ic * ICW
                    nm = nps.tile([65, ICW], f32, tag="num")
                    gs = [None] * JT
                    for jt in range(JT):
                        jsl = slice(jt * 128, (jt + 1) * 128)
                        sp = sps.tile([128, ICW], f32, tag="scores")
                        for q in range(2):
                            qsl = slice(q * 512, (q + 1) * 512)
                            nc.tensor.matmul(
                                sp[:, qsl], lhsT=qT[dt][rows, jsl],
                                rhs=qT[dt][rows, i0 + q * 512 : i0 + (q + 1) * 512],
                                start=True, stop=True)
                        g = gp.tile([128, ICW], f32r, tag="gtile")
                        col = h * JT + jt
                        nc.scalar.activation(out=g, in_=sp, func=EXP,
                                             bias=aa[:, col : col + 1],
                                             scale=2.0 * SCALE)
                        gs[jt] = g
                        # one-step software skew: num(jt-1) after S(jt)/exp(jt)
                        if jt > 0:
                            for q in range(2):
                                qsl = slice(q * 512, (q + 1) * 512)
                                nc.tensor.matmul(nm[:, qsl], lhsT=v_aug[:, jt - 1, vsl],
                                                 rhs=gs[jt - 1][:, qsl],
                                                 start=(jt == 1), stop=False)
                    for q in range(2):
                        qsl = slice(q * 512, (q + 1) * 512)
                        nc.tensor.matmul(nm[:, qsl], lhsT=v_aug[:, JT - 1, vsl],
                                         rhs=gs[JT - 1][:, qsl],
                                         start=False, stop=True)
                    # normalize: ot[rows, i0:i0+ICW] = nm[0:64] / nm[64]
                    rd = nrm.tile([1, ICW], f32, tag="rden", name="rden")
                    nc.vector.reciprocal(rd, nm[64:65, :])
                    rdb = nrm.tile([64, ICW], f32, tag="rdenb", name="rdenb")
                    nc.gpsimd.partition_broadcast(rdb, rd)
                    nc.vector.tensor_mul(ot[dt][rows, i0 : i0 + ICW],
                                         nm[0:64, :], rdb)

        # ---- phase 3: output projection ----
        with tc.tile_pool(name="ops", bufs=2, space="PSUM") as ops, \
             tc.tile_pool(name="osb", bufs=3) as osb:
            for it in range(JT):
                isl = slice(it * 128, (it + 1) * 128)
                ps = ops.tile([128, 1024], f32, tag="oproj")
                for ock in range(2):
                    osl = slice(ock * 512, (ock + 1) * 512)
                    for dlt in range(4):
                        nc.tensor.matmul(ps[:, osl], lhsT=ot[dlt][:, isl],
                                         rhs=wo_sb[dlt][:, osl],
                                         start=(dlt == 0), stop=(dlt == 3))
                ob = osb.tile([128, 1024], f32, tag="obuf", name="obuf")
                nc.vector.tensor_copy(ob, ps)
                nc.sync.dma_start(out=part[isl, :], in_=ob)

        p2.release()

    nc.compile()
    return nc


def _get_nc():
    if "nc" not in _CACHE:
        _CACHE["nc"] = _build_nc()
    return _CACHE["nc"]


def make_in_maps(x, Wq, bq, Wv, bv, Wo, bo):
    x = np.asarray(x, dtype=np.float32)
    Wq = np.asarray(Wq, dtype=np.float32)
    bq = np.asarray(bq, dtype=np.float32)
    Wv = np.asarray(Wv, dtype=np.float32)
    bv = np.asarray(bv, dtype=np.float32)
    Wo = np.asarray(Wo, dtype=np.float32)
    in_maps = []
    for c in range(NCORES):
        b, g = divmod(c, 2)
        gsl = slice(g * DL, (g + 1) * DL)
        xTa = np.concatenate([np.ascontiguousarray(x[b].T),
                              np.ones((1, N), np.float32)], axis=0)
        wqa = np.concatenate([Wq[:, gsl], bq[gsl][None, :]], axis=0)
        wva = np.concatenate([Wv[:, gsl], bv[gsl][None, :]], axis=0)
        in_maps.append({
            "xTa": np.ascontiguousarray(xTa),
            "wqa": np.ascontiguousarray(wqa),
            "wva": np.ascontiguousarray(wva),
            "wo": np.ascontiguousarray(Wo[gsl, :]),
        })
    return in_maps


def combine_parts(parts, bo):
    bo = np.asarray(bo, dtype=np.float32)
    out = np.empty((B, N, D), np.float32)
    for b in range(B):
        out[b] = parts[2 * b] + parts[2 * b + 1] + bo
    return out


def kernel(x, Wq, bq, Wv, bv, Wo, bo):
    from concourse.bass_utils import run_bass_kernel_spmd

    nc = _get_nc()
    in_maps = make_in_maps(x, Wq, bq, Wv, bv, Wo, bo)
    res = run_bass_kernel_spmd(nc, in_maps, core_ids=list(range(NCORES)))
    parts = [r["part"] for r in res.results]
    return combine_parts(parts, bo)



# revision 5
# speedup vs baseline: 1.2172x; 1.2172x over previous
"""L2 self-attention (q==k) Bass/Tile kernel for 8 TRN2 NeuronCores.

Sharding: core c = 2*b + g handles batch b and head-group g (8 of 16 heads).
Each core computes the partial output  attn_out_bg @ Wo[g*512:(g+1)*512, :].
Host sums the two partials per batch and adds bo.

Math per head (s = DIM_HEAD**-0.5):
  sim_ij = -s*||q_i - q_j||^2 = 2s*AB_ij - s*AA_i - s*AA_j
  softmax rows are invariant to the per-row constant  -s*AA_i, so
  P_ij = exp(2s*AB_ij - s*AA_j) / sum_j exp(2s*AB_ij - s*AA_j)
  Exponent is <= s*AA_i + bounded (2ab <= a^2+b^2), no max-subtraction needed.
  out = P @ v ; den folded in as a 65th (ones) column of v.

v2: all matmul operands in bf16 (fp32 PSUM accumulation).  fp32r moving
operands stream at half rate and keep the PE HAM clock gate at K=4/8
(1.2 GHz) for the whole attention phase; bf16 streams at 1 row/cycle and
holds 2.4 GHz.  The -s*AA_j exp bias is folded into the scores matmul as a
65th contraction row (stationary side carries AA_j/2, moving side carries
-1), which kills the masked-diag AA pass entirely.  AA itself comes from a
scalar-engine Square (scale 1/sqrt(2)) + gpsimd partition_all_reduce.
The softmax denominator reciprocal uses the fast approx DVE op.
"""

import numpy as np

B, N, D = 4, 2048, 1024
HEADS, DIM_HEAD = 16, 64
INNER = HEADS * DIM_HEAD
SCALE = DIM_HEAD ** -0.5

NCORES = 8
NH = 8            # heads per core
DL = NH * DIM_HEAD  # 512 local inner dims
KT = D // 128     # 8 full contraction tiles for projections
JT = N // 128     # 16 key tiles
IC = 2            # i-chunks of 1024 columns
ICW = N // IC     # 1024

_CACHE = {}


def _build_nc():
    import concourse.bacc as bacc
    import concourse.bass as bass
    import concourse.bass_isa as bass_isa
    import concourse.mybir as mybir
    import concourse.tile as tile

    f32 = mybir.dt.float32
    bf16 = mybir.dt.bfloat16
    EXP = mybir.ActivationFunctionType.Exp
    SQUARE = mybir.ActivationFunctionType.Square
    COPY = mybir.ActivationFunctionType.Copy

    nc = bacc.Bacc("TRN2", target_bir_lowering=False, debug=False,
                   num_devices=NCORES)

    xTa_d = nc.dram_tensor("xTa", [D + 1, N], bf16, kind="ExternalInput")
    wqa_d = nc.dram_tensor("wqa", [D + 1, DL], bf16, kind="ExternalInput")
    wva_d = nc.dram_tensor("wva", [D + 1, DL], bf16, kind="ExternalInput")
    wo_d = nc.dram_tensor("wo", [DL, D], bf16, kind="ExternalInput")
    part_d = nc.dram_tensor("part", [N, D], f32, kind="ExternalOutput")
    xTa = xTa_d.ap()
    wqa = wqa_d.ap()
    wva = wva_d.ap()
    wo_ap = wo_d.ap()
    part = part_d.ap()

    with tile.TileContext(nc) as tc, \
         tc.tile_pool(name="persist", bufs=1) as persist:
        # ---- persistent tensors (whole-kernel lifetime) ----
        # per-head stationary q: rows 0..63 = q_h, row 64 = AA_h/2
        qS = [persist.tile([65, N], bf16, tag=f"qS{h}", name=f"qS{h}")
              for h in range(NH)]
        # per-head moving q: rows 0..63 = q_h, row 64 = -1
        qM = [persist.tile([65, N], bf16, tag=f"qM{h}", name=f"qM{h}")
              for h in range(NH)]
        v_aug = persist.tile([128, JT, NH * 65], bf16, tag="v_aug", name="v_aug")

        ones1 = persist.tile([128, JT, 1], bf16, tag="ones1", name="ones1")
        nc.vector.memset(ones1, 1.0)
        for h in range(NH):
            nc.vector.tensor_copy(v_aug[:, :, h * 65 + 64 : h * 65 + 65], ones1)
            nc.vector.memset(qM[h][64:65, :], -1.0)

        # ---- phase 1: projections ----
        with tc.tile_pool(name="pin", bufs=1) as pin, \
             tc.tile_pool(name="sqp", bufs=2) as sqp, \
             tc.tile_pool(name="aap", bufs=2) as aap:
            xt = [pin.tile([128, N], bf16, tag=f"xt{k}", name=f"xt{k}") for k in range(KT)]
            xt.append(pin.tile([1, N], bf16, tag="xt_ones", name="xt_ones"))
            wq = [pin.tile([128, DL], bf16, tag=f"wq{k}", name=f"wq{k}") for k in range(KT)]
            wq.append(pin.tile([1, DL], bf16, tag="wq_b", name="wq_b"))
            wv = [pin.tile([128, DL], bf16, tag=f"wv{k}", name=f"wv{k}") for k in range(KT)]
            wv.append(pin.tile([1, DL], bf16, tag="wv_b", name="wv_b"))
            for k in range(KT):
                sl = slice(k * 128, (k + 1) * 128)
                nc.sync.dma_start(out=xt[k], in_=xTa[sl, :])
                nc.sync.dma_start(out=wq[k], in_=wqa[sl, :])
                nc.sync.dma_start(out=wv[k], in_=wva[sl, :])
            nc.sync.dma_start(out=xt[KT], in_=xTa[D : D + 1, :])
            nc.sync.dma_start(out=wq[KT], in_=wqa[D : D + 1, :])
            nc.sync.dma_start(out=wv[KT], in_=wva[D : D + 1, :])

            # q[d, i] per dt tile: lhsT = wqa[:, d-tile], rhs = xTa[:, i-chunk]
            with tc.tile_pool(name="qps", bufs=2, space="PSUM") as qps:
                for dt in range(4):
                    ps = qps.tile([128, N], f32, tag="qproj")
                    dsl = slice(dt * 128, (dt + 1) * 128)
                    for k in range(KT + 1):
                        for nck in range(4):
                            nsl = slice(nck * 512, (nck + 1) * 512)
                            nc.tensor.matmul(ps[:, nsl], lhsT=wq[k][:, dsl],
                                             rhs=xt[k][:, nsl],
                                             start=(k == 0), stop=(k == KT))
                    h0, h1 = 2 * dt, 2 * dt + 1
                    # split heads into per-head stationary/moving tiles (bf16)
                    nc.vector.tensor_copy(qS[h0][0:64, :], ps[0:64, :])
                    nc.vector.tensor_copy(qS[h1][0:64, :], ps[64:128, :])
                    nc.scalar.activation(qM[h0][0:64, :], ps[0:64, :], COPY)
                    nc.scalar.activation(qM[h1][0:64, :], ps[64:128, :], COPY)
                    # sq = q^2 / 2 (Square of q/sqrt(2)), one tile per head:
                    # partition_all_reduce ignores input partition offsets, so
                    # each head's squares must start at partition 0.
                    for hh, rows in ((h0, slice(0, 64)), (h1, slice(64, 128))):
                        sq = sqp.tile([64, N], bf16, tag="sq")
                        nc.scalar.activation(sq, ps[rows, :], SQUARE,
                                             scale=0.7071067811865476)
                        aat = aap.tile([64, N], bf16, tag="aat")
                        nc.gpsimd.partition_all_reduce(
                            aat, sq, channels=64,
                            reduce_op=bass_isa.ReduceOp.add)
                        nc.vector.tensor_copy(qS[hh][64:65, :], aat[0:1, :])

            # v[i, d] : lhsT = xTa[:, i-tile], rhs = wva ; scatter into v_aug
            with tc.tile_pool(name="vps", bufs=4, space="PSUM") as vps:
                for it in range(JT):
                    ps = vps.tile([128, DL], f32, tag="vproj")
                    isl = slice(it * 128, (it + 1) * 128)
                    for k in range(KT + 1):
                        nc.tensor.matmul(ps, lhsT=xt[k][:, isl], rhs=wv[k],
                                         start=(k == 0), stop=(k == KT))
                    src = ps.rearrange("p (h w) -> p h w", w=64)
                    dst = v_aug[:, it, :].rearrange("p (h w) -> p h w", w=65)
                    nc.vector.tensor_copy(dst[:, :, 0:64], src)

        # allocated after the projection pool closes so phase-1 SBUF peak
        # (xt/wq/wv tiles) and these never coexist in the address map
        p2 = tc.alloc_tile_pool(name="persist2", bufs=1)
        ot = [p2.tile([128, N], bf16, tag=f"ot{t}", name=f"ot{t}")
              for t in range(4)]
        wo_sb = [p2.tile([128, D], bf16, tag=f"wo{t}", name=f"wo{t}")
                 for t in range(4)]
        for t in range(4):
            nc.sync.dma_start(out=wo_sb[t], in_=wo_ap[t * 128 : (t + 1) * 128, :])

        # ---- phase 2: attention per head ----
        # scores K=65: sp = q_h^T q_h - AA_j/2 ; exp(2s*sp) is the softmax
        # numerator with the j-bias folded in.
        with tc.tile_pool(name="sps", bufs=2, space="PSUM") as sps, \
             tc.tile_pool(name="nps", bufs=2, space="PSUM") as nps, \
             tc.tile_pool(name="gp", bufs=3) as gp, \
             tc.tile_pool(name="nrm", bufs=2) as nrm:
            for h in range(NH):
                dt, half = divmod(h, 2)
                rows = slice(half * 64, half * 64 + 64)
                vsl = slice(h * 65, (h + 1) * 65)
                for ic in range(IC):
                    i0 = ic * ICW
                    nm = nps.tile([65, ICW], f32, tag="num")
                    gs = [None] * JT
                    for jt in range(JT):
                        jsl = slice(jt * 128, (jt + 1) * 128)
                        sp = sps.tile([128, ICW], f32, tag="scores")
                        for q in range(2):
                            qsl = slice(q * 512, (q + 1) * 512)
                            nc.tensor.matmul(
                                sp[:, qsl], lhsT=qS[h][:, jsl],
                                rhs=qM[h][:, i0 + q * 512 : i0 + (q + 1) * 512],
                                start=True, stop=True)
                        g = gp.tile([128, ICW], bf16, tag="gtile")
                        nc.scalar.activation(out=g, in_=sp, func=EXP,
                                             scale=2.0 * SCALE)
                        gs[jt] = g
                        # one-step software skew: num(jt-1) after S(jt)/exp(jt)
                        if jt > 0:
                            for q in range(2):
                                qsl = slice(q * 512, (q + 1) * 512)
                                nc.tensor.matmul(nm[:, qsl], lhsT=v_aug[:, jt - 1, vsl],
                                                 rhs=gs[jt - 1][:, qsl],
                                                 start=(jt == 1), stop=False)
                    for q in range(2):
                        qsl = slice(q * 512, (q + 1) * 512)
                        nc.tensor.matmul(nm[:, qsl], lhsT=v_aug[:, JT - 1, vsl],
                                         rhs=gs[JT - 1][:, qsl],
                                         start=False, stop=True)
                    # normalize: ot[rows, i0:i0+ICW] = nm[0:64] / nm[64]
                    # reciprocal_approx_fast misreads PSUM inputs — stage the
                    # denominator row through SBUF first.
                    dsb = nrm.tile([1, ICW], f32, tag="dsb", name="dsb")
                    nc.vector.tensor_copy(dsb, nm[64:65, :])
                    rd = nrm.tile([1, ICW], f32, tag="rden", name="rden")
                    nc.vector.reciprocal_approx_fast(out=rd, in_=dsb)
                    rdb = nrm.tile([64, ICW], f32, tag="rdenb", name="rdenb")
                    nc.gpsimd.partition_broadcast(rdb, rd)
                    nc.vector.tensor_mul(ot[dt][rows, i0 : i0 + ICW],
                                         nm[0:64, :], rdb)

        # ---- phase 3: output projection ----
        with tc.tile_pool(name="ops", bufs=2, space="PSUM") as ops, \
             tc.tile_pool(name="osb", bufs=3) as osb:
            for it in range(JT):
                isl = slice(it * 128, (it + 1) * 128)
                ps = ops.tile([128, 1024], f32, tag="oproj")
                for ock in range(2):
                    osl = slice(ock * 512, (ock + 1) * 512)
                    for dlt in range(4):
                        nc.tensor.matmul(ps[:, osl], lhsT=ot[dlt][:, isl],
                                         rhs=wo_sb[dlt][:, osl],
                                         start=(dlt == 0), stop=(dlt == 3))
                ob = osb.tile([128, 1024], f32, tag="obuf", name="obuf")
                nc.vector.tensor_copy(ob, ps)
                nc.sync.dma_start(out=part[isl, :], in_=ob)

        p2.release()

    nc.compile()
    return nc


def _get_nc():
    if "nc" not in _CACHE:
        _CACHE["nc"] = _build_nc()
    return _CACHE["nc"]


def make_in_maps(x, Wq, bq, Wv, bv, Wo, bo):
    from ml_dtypes import bfloat16

    x = np.asarray(x, dtype=np.float32)
    Wq = np.asarray(Wq, dtype=np.float32)
    bq = np.asarray(bq, dtype=np.float32)
    Wv = np.asarray(Wv, dtype=np.float32)
    bv = np.asarray(bv, dtype=np.float32)
    Wo = np.asarray(Wo, dtype=np.float32)
    in_maps = []
    for c in range(NCORES):
        b, g = divmod(c, 2)
        gsl = slice(g * DL, (g + 1) * DL)
        xTa = np.concatenate([np.ascontiguousarray(x[b].T),
                              np.ones((1, N), np.float32)], axis=0)
        wqa = np.concatenate([Wq[:, gsl], bq[gsl][None, :]], axis=0)
        wva = np.concatenate([Wv[:, gsl], bv[gsl][None, :]], axis=0)
        in_maps.append({
            "xTa": np.ascontiguousarray(xTa).astype(bfloat16),
            "wqa": np.ascontiguousarray(wqa).astype(bfloat16),
            "wva": np.ascontiguousarray(wva).astype(bfloat16),
            "wo": np.ascontiguousarray(Wo[gsl, :]).astype(bfloat16),
        })
    return in_maps


def combine_parts(parts, bo):
    bo = np.asarray(bo, dtype=np.float32)
    out = np.empty((B, N, D), np.float32)
    for b in range(B):
        out[b] = parts[2 * b] + parts[2 * b + 1] + bo
    return out


def kernel(x, Wq, bq, Wv, bv, Wo, bo):
    from concourse.bass_utils import run_bass_kernel_spmd

    nc = _get_nc()
    in_maps = make_in_maps(x, Wq, bq, Wv, bv, Wo, bo)
    res = run_bass_kernel_spmd(nc, in_maps, core_ids=list(range(NCORES)))
    parts = [r["part"] for r in res.results]
    return combine_parts(parts, bo)


# revision 6
# speedup vs baseline: 1.7018x; 1.3981x over previous
"""L2 self-attention (q==k) Bass/Tile kernel for 8 TRN2 NeuronCores.

Sharding: core c = 2*b + g handles batch b and head-group g (8 of 16 heads).
Each core computes the partial output  attn_out_bg @ Wo[g*512:(g+1)*512, :].
Host sums the two partials per batch and adds bo.

Math per head (s = DIM_HEAD**-0.5):
  sim_ij = -s*||q_i - q_j||^2 = 2s*AB_ij - s*AA_i - s*AA_j
  softmax rows are invariant to the per-row constant  -s*AA_i, so
  P_ij = exp(2s*AB_ij - s*AA_j) / sum_j exp(2s*AB_ij - s*AA_j)
  Exponent is bounded (2ab <= a^2+b^2), no max-subtraction needed.
  out = P @ v ; den folded in as a 65th (ones) column of v.

v3: all matmul operands 16-bit (fp32 PSUM accumulation).  fp32r moving
operands stream at half rate and keep the PE HAM clock gate at K=4/8
(1.2 GHz) for the whole attention phase; 16-bit streams 1 row/cycle and
holds 2.4 GHz.  The -s*AA_j term is folded into the scores matmul as a
65th contraction row: stationary qS row 64 = AA_j (computed by a
ones-vector PE matmul over squared q), moving qM row 64 = -0.5, and the
exp applies scale 2s, giving exp(2s*AB - s*AA_j) with no bias lookup.
Softmax denominator via reciprocal_approx_fast (SBUF-staged; the approx
op misreads PSUM) + gpsimd partition broadcast.
"""

import numpy as np

B, N, D = 4, 2048, 1024
HEADS, DIM_HEAD = 16, 64
INNER = HEADS * DIM_HEAD
SCALE = DIM_HEAD ** -0.5

NCORES = 8
NH = 8            # heads per core
DL = NH * DIM_HEAD  # 512 local inner dims
KT = D // 128     # 8 full contraction tiles for projections
JT = N // 128     # 16 key tiles
IC = 2            # i-chunks of 1024 columns
ICW = N // IC     # 1024

_CACHE = {}


def _build_nc():
    import concourse.bacc as bacc
    import concourse.bass as bass
    import concourse.mybir as mybir
    import concourse.tile as tile

    f32 = mybir.dt.float32
    bf16 = mybir.dt.bfloat16
    f16 = mybir.dt.float16
    EXP = mybir.ActivationFunctionType.Exp
    SQUARE = mybir.ActivationFunctionType.Square
    COPY = mybir.ActivationFunctionType.Copy

    nc = bacc.Bacc("TRN2", target_bir_lowering=False, debug=False,
                   num_devices=NCORES)

    xTa_d = nc.dram_tensor("xTa", [D + 1, N], bf16, kind="ExternalInput")
    wqa_d = nc.dram_tensor("wqa", [D + 1, DL], bf16, kind="ExternalInput")
    wva_d = nc.dram_tensor("wva", [D + 1, DL], bf16, kind="ExternalInput")
    wo_d = nc.dram_tensor("wo", [DL, D], bf16, kind="ExternalInput")
    part_d = nc.dram_tensor("part", [N, D], f32, kind="ExternalOutput")
    xTa = xTa_d.ap()
    wqa = wqa_d.ap()
    wva = wva_d.ap()
    wo_ap = wo_d.ap()
    part = part_d.ap()

    with tile.TileContext(nc) as tc, \
         tc.tile_pool(name="persist", bufs=1) as persist:
        # ---- persistent tensors (whole-kernel lifetime) ----
        # per-head stationary q: rows 0..63 = q_h, row 64 = AA_h
        qS = [persist.tile([65, N], bf16, tag=f"qS{h}", name=f"qS{h}")
              for h in range(NH)]
        # per-head moving q: rows 0..63 = q_h, row 64 = -0.5
        qM = [persist.tile([65, N], bf16, tag=f"qM{h}", name=f"qM{h}")
              for h in range(NH)]
        v_aug = persist.tile([128, JT, NH * 65], f16, tag="v_aug", name="v_aug")
        ones64 = persist.tile([64, 1], bf16, tag="ones64", name="ones64")
        nc.vector.memset(ones64, 1.0)

        ones1 = persist.tile([128, JT, 1], f16, tag="ones1", name="ones1")
        nc.vector.memset(ones1, 1.0)
        for h in range(NH):
            nc.vector.tensor_copy(v_aug[:, :, h * 65 + 64 : h * 65 + 65], ones1)
            nc.vector.memset(qM[h][64:65, :], -0.5)

        # ---- phase 1: projections ----
        with tc.tile_pool(name="pin", bufs=1) as pin, \
             tc.tile_pool(name="sqp", bufs=2) as sqp:
            xt = [pin.tile([128, N], bf16, tag=f"xt{k}", name=f"xt{k}") for k in range(KT)]
            xt.append(pin.tile([1, N], bf16, tag="xt_ones", name="xt_ones"))
            wq = [pin.tile([128, DL], bf16, tag=f"wq{k}", name=f"wq{k}") for k in range(KT)]
            wq.append(pin.tile([1, DL], bf16, tag="wq_b", name="wq_b"))
            wv = [pin.tile([128, DL], bf16, tag=f"wv{k}", name=f"wv{k}") for k in range(KT)]
            wv.append(pin.tile([1, DL], bf16, tag="wv_b", name="wv_b"))
            for k in range(KT):
                sl = slice(k * 128, (k + 1) * 128)
                nc.sync.dma_start(out=xt[k], in_=xTa[sl, :])
                nc.sync.dma_start(out=wq[k], in_=wqa[sl, :])
                nc.sync.dma_start(out=wv[k], in_=wva[sl, :])
            nc.sync.dma_start(out=xt[KT], in_=xTa[D : D + 1, :])
            nc.sync.dma_start(out=wq[KT], in_=wqa[D : D + 1, :])
            nc.sync.dma_start(out=wv[KT], in_=wva[D : D + 1, :])

            # q[d, i] per dt tile: lhsT = wqa[:, d-tile], rhs = xTa[:, i-chunk]
            with tc.tile_pool(name="qps", bufs=2, space="PSUM") as qps:
                for dt in range(4):
                    ps = qps.tile([128, N], f32, tag="qproj")
                    dsl = slice(dt * 128, (dt + 1) * 128)
                    for k in range(KT + 1):
                        for nck in range(4):
                            nsl = slice(nck * 512, (nck + 1) * 512)
                            nc.tensor.matmul(ps[:, nsl], lhsT=wq[k][:, dsl],
                                             rhs=xt[k][:, nsl],
                                             start=(k == 0), stop=(k == KT))
                    h0, h1 = 2 * dt, 2 * dt + 1
                    # split heads into per-head stationary/moving tiles (bf16)
                    nc.vector.tensor_copy(qS[h0][0:64, :], ps[0:64, :])
                    nc.vector.tensor_copy(qS[h1][0:64, :], ps[64:128, :])
                    nc.scalar.activation(qM[h0][0:64, :], ps[0:64, :], COPY)
                    nc.scalar.activation(qM[h1][0:64, :], ps[64:128, :], COPY)

            # v[i, d] : lhsT = xTa[:, i-tile], rhs = wva ; scatter into v_aug
            # AA rows: sq = q_h^2 (from the bf16 qS rows so rounding matches
            # the scores matmul operands), then ones^T @ sq via the PE.
            with tc.tile_pool(name="vps", bufs=4, space="PSUM") as vps, \
                 tc.tile_pool(name="aaps", bufs=4, space="PSUM") as aaps:
                for h in range(NH):
                    sq = sqp.tile([64, N], bf16, tag="sq")
                    nc.scalar.activation(sq, qS[h][0:64, :], SQUARE)
                    for c4 in range(4):
                        csl = slice(c4 * 512, (c4 + 1) * 512)
                        aps = aaps.tile([1, 512], f32, tag="aa")
                        nc.tensor.matmul(aps, lhsT=ones64, rhs=sq[:, csl],
                                         start=True, stop=True)
                        nc.scalar.activation(qS[h][64:65, csl], aps, COPY)
                for it in range(JT):
                    ps = vps.tile([128, DL], f32, tag="vproj")
                    isl = slice(it * 128, (it + 1) * 128)
                    for k in range(KT + 1):
                        nc.tensor.matmul(ps, lhsT=xt[k][:, isl], rhs=wv[k],
                                         start=(k == 0), stop=(k == KT))
                    src = ps.rearrange("p (h w) -> p h w", w=64)
                    dst = v_aug[:, it, :].rearrange("p (h w) -> p h w", w=65)
                    nc.vector.tensor_copy(dst[:, :, 0:64], src)

        # allocated after the projection pool closes so phase-1 SBUF peak
        # (xt/wq/wv tiles) and these never coexist in the address map
        p2 = tc.alloc_tile_pool(name="persist2", bufs=1)
        ot = [p2.tile([128, N], bf16, tag=f"ot{t}", name=f"ot{t}")
              for t in range(4)]
        wo_sb = [p2.tile([128, D], bf16, tag=f"wo{t}", name=f"wo{t}")
                 for t in range(4)]
        for t in range(4):
            nc.sync.dma_start(out=wo_sb[t], in_=wo_ap[t * 128 : (t + 1) * 128, :])

        # ---- phase 2: attention per head ----
        # scores K=65: sp = q_h^T q_h - AA_j/2 ; exp(2s*sp) is the softmax
        # numerator with the j-bias folded in.
        with tc.tile_pool(name="sps", bufs=2, space="PSUM") as sps, \
             tc.tile_pool(name="nps", bufs=2, space="PSUM") as nps, \
             tc.tile_pool(name="gp", bufs=3) as gp, \
             tc.tile_pool(name="nrm", bufs=2) as nrm:
            for h in range(NH):
                dt, half = divmod(h, 2)
                rows = slice(half * 64, half * 64 + 64)
                vsl = slice(h * 65, (h + 1) * 65)
                for ic in range(IC):
                    i0 = ic * ICW
                    nm = nps.tile([65, ICW], f32, tag="num")
                    gs = [None] * JT
                    for jt in range(JT):
                        jsl = slice(jt * 128, (jt + 1) * 128)
                        sp = sps.tile([128, ICW], f32, tag="scores")
                        for q in range(2):
                            qsl = slice(q * 512, (q + 1) * 512)
                            nc.tensor.matmul(
                                sp[:, qsl], lhsT=qS[h][:, jsl],
                                rhs=qM[h][:, i0 + q * 512 : i0 + (q + 1) * 512],
                                start=True, stop=True)
                        g = gp.tile([128, ICW], f16, tag="gtile")
                        nc.scalar.activation(out=g, in_=sp, func=EXP,
                                             scale=2.0 * SCALE)
                        gs[jt] = g
                        # one-step software skew: num(jt-1) after S(jt)/exp(jt)
                        if jt > 0:
                            for q in range(2):
                                qsl = slice(q * 512, (q + 1) * 512)
                                nc.tensor.matmul(nm[:, qsl], lhsT=v_aug[:, jt - 1, vsl],
                                                 rhs=gs[jt - 1][:, qsl],
                                                 start=(jt == 1), stop=False)
                    for q in range(2):
                        qsl = slice(q * 512, (q + 1) * 512)
                        nc.tensor.matmul(nm[:, qsl], lhsT=v_aug[:, JT - 1, vsl],
                                         rhs=gs[JT - 1][:, qsl],
                                         start=False, stop=True)
                    # normalize: ot[rows, i0:i0+ICW] = nm[0:64] / nm[64]
                    # (reciprocal_approx_fast misreads PSUM - stage via SBUF)
                    dsb = nrm.tile([1, ICW], f32, tag="dsb", name="dsb")
                    nc.vector.tensor_copy(dsb, nm[64:65, :])
                    rd = nrm.tile([1, ICW], f32, tag="rden", name="rden")
                    nc.vector.reciprocal_approx_fast(out=rd, in_=dsb)
                    rdb = nrm.tile([64, ICW], f32, tag="rdenb", name="rdenb")
                    nc.gpsimd.partition_broadcast(rdb, rd)
                    nc.vector.tensor_mul(ot[dt][rows, i0 : i0 + ICW],
                                         nm[0:64, :], rdb)

        # ---- phase 3: output projection ----
        with tc.tile_pool(name="ops", bufs=2, space="PSUM") as ops, \
             tc.tile_pool(name="osb", bufs=3) as osb:
            for it in range(JT):
                isl = slice(it * 128, (it + 1) * 128)
                ps = ops.tile([128, 1024], f32, tag="oproj")
                for ock in range(2):
                    osl = slice(ock * 512, (ock + 1) * 512)
                    for dlt in range(4):
                        nc.tensor.matmul(ps[:, osl], lhsT=ot[dlt][:, isl],
                                         rhs=wo_sb[dlt][:, osl],
                                         start=(dlt == 0), stop=(dlt == 3))
                ob = osb.tile([128, 1024], f32, tag="obuf", name="obuf")
                nc.vector.tensor_copy(ob, ps)
                nc.sync.dma_start(out=part[isl, :], in_=ob)

        p2.release()

    nc.compile()
    return nc


def _get_nc():
    if "nc" not in _CACHE:
        _CACHE["nc"] = _build_nc()
    return _CACHE["nc"]


def make_in_maps(x, Wq, bq, Wv, bv, Wo, bo):
    from ml_dtypes import bfloat16

    x = np.asarray(x, dtype=np.float32)
    Wq = np.asarray(Wq, dtype=np.float32)
    bq = np.asarray(bq, dtype=np.float32)
    Wv = np.asarray(Wv, dtype=np.float32)
    bv = np.asarray(bv, dtype=np.float32)
    Wo = np.asarray(Wo, dtype=np.float32)
    in_maps = []
    for c in range(NCORES):
        b, g = divmod(c, 2)
        gsl = slice(g * DL, (g + 1) * DL)
        xTa = np.concatenate([np.ascontiguousarray(x[b].T),
                              np.ones((1, N), np.float32)], axis=0)
        wqa = np.concatenate([Wq[:, gsl], bq[gsl][None, :]], axis=0)
        wva = np.concatenate([Wv[:, gsl], bv[gsl][None, :]], axis=0)
        in_maps.append({
            "xTa": np.ascontiguousarray(xTa).astype(bfloat16),
            "wqa": np.ascontiguousarray(wqa).astype(bfloat16),
            "wva": np.ascontiguousarray(wva).astype(bfloat16),
            "wo": np.ascontiguousarray(Wo[gsl, :]).astype(bfloat16),
        })
    return in_maps


def combine_parts(parts, bo):
    bo = np.asarray(bo, dtype=np.float32)
    out = np.empty((B, N, D), np.float32)
    for b in range(B):
        out[b] = parts[2 * b] + parts[2 * b + 1] + bo
    return out


def kernel(x, Wq, bq, Wv, bv, Wo, bo):
    from concourse.bass_utils import run_bass_kernel_spmd

    nc = _get_nc()
    in_maps = make_in_maps(x, Wq, bq, Wv, bv, Wo, bo)
    res = run_bass_kernel_spmd(nc, in_maps, core_ids=list(range(NCORES)))
    parts = [r["part"] for r in res.results]
    return combine_parts(parts, bo)
